# revision 11
# baseline (speedup 1.0000x reference)
"""Trainium2 Bass kernel for tied-row MSA attention (nn_Attention_52329881535135).

Strategy (8 NeuronCores, one chip):
  - Shard the MSA row dim r (leading b*r=256) across the 8 cores: 32 rows each.
  - Each core: q/k projections for its rows into transposed per-head layout,
    accumulate the row-tied logits dots[h,j,i] = sum_r k_r^T q_r in PSUM/SBUF.
  - AllReduce the 8.4MB fp32 logits across the 8 cores (sum over all 256 rows).
  - Every core computes the (replicated) softmax, then out = attn @ v for its
    own 32 rows, followed by the output projection; host concatenates shards.

  Mask bookkeeping (has_rows / num_rows / mask_any) is computed on the host at
  call time and folded into the weights / an additive column bias, so the
  device graph only does dense matmuls.

  Compute dtype: bf16 matmul inputs, fp32 PSUM accumulation, fp32 logits
  and AllReduce, fp32 softmax normalization.
"""

import sys

sys.path.insert(0, "/opt/trn_rl_repo")

import numpy as np

B, R, N, D, H, DH = 1, 256, 512, 256, 8, 64
INNER = H * DH
NCORES = 8
R_LOC = R // NCORES  # 32 rows per core
P = 128
NPT = N // P  # 4 position tiles
NJT = N // P  # 4 j tiles
NDT = D // P  # 2 d tiles
NHT = INNER // P  # 4 hd tiles
V_PREFETCH = 6  # pairs whose v-projection is emitted before softmax (AR overlap)

_graph_cache = {}
_result_cache = {}


def _build(
    separate_xq: bool,
    r_loc: int = R_LOC,
    n_cores: int = NCORES,
    do_finalize: bool = True,
):
    from contextlib import ExitStack

    from concourse import bacc, mybir, tile
    from concourse.masks import make_identity

    f32 = mybir.dt.float32
    bf16 = mybir.dt.bfloat16
    AF = mybir.ActivationFunctionType
    ALU = mybir.AluOpType

    nc = bacc.Bacc(
        "TRN2", target_bir_lowering=False, debug=False, num_devices=n_cores
    )

    x_ext = nc.declare_dram_parameter("x", [r_loc, N, D], f32, isOutput=False)
    if separate_xq:
        xq_ext = nc.declare_dram_parameter("xq", [r_loc, N, D], f32, isOutput=False)
    else:
        xq_ext = x_ext
    wq_ext = nc.declare_dram_parameter("Wq", [D, INNER], f32, isOutput=False)
    wk_ext = nc.declare_dram_parameter("Wk", [D, INNER], f32, isOutput=False)
    wv_ext = nc.declare_dram_parameter("Wv", [D, INNER], f32, isOutput=False)
    wo_ext = nc.declare_dram_parameter("Wo", [INNER, D], f32, isOutput=False)
    bo_ext = nc.declare_dram_parameter("bo", [D], f32, isOutput=False)
    jb_ext = nc.declare_dram_parameter("jbias", [NJT, P], f32, isOutput=False)
    out_ext = nc.declare_dram_parameter("out", [r_loc, N, D], f32, isOutput=True)

    cc_shape = [P, H, NJT, N]
    cc_in = nc.dram_tensor("cc_in", cc_shape, f32)
    cc_out = nc.dram_tensor(
        "cc_out", cc_shape, f32, addr_space="Shared" if n_cores > 4 else "Local"
    )

    PAIRS = r_loc // 2
    # pair groups of <=4 pairs (8 rows): bounds SBUF held q/k projections
    groups = [list(range(g, min(g + 4, PAIRS))) for g in range(0, PAIRS, 4)]
    GMAX = max(len(g) for g in groups)

    with tile.TileContext(nc) as tc, ExitStack() as top:
        consts = top.enter_context(tc.tile_pool(name="consts", bufs=1))
        tpsum = top.enter_context(tc.tile_pool(name="tpsum", bufs=2, space="PSUM"))
        xin_pool = top.enter_context(tc.tile_pool(name="xin", bufs=3))
        xt_pool = top.enter_context(tc.tile_pool(name="xt", bufs=3))

        # --- constants / weights (resident in SBUF, cast to bf16 on load) ---
        wq_sb = consts.tile([P, NDT, INNER], bf16, name="wq_sb")
        nc.gpsimd.dma_start(wq_sb[:], wq_ext.rearrange("(o p) f -> p o f", p=P))
        wk_sb = consts.tile([P, NDT, INNER], bf16, name="wk_sb")
        nc.gpsimd.dma_start(wk_sb[:], wk_ext.rearrange("(o p) f -> p o f", p=P))
        wv_sb = consts.tile([P, NDT, INNER], bf16, name="wv_sb")
        nc.gpsimd.dma_start(wv_sb[:], wv_ext.rearrange("(o p) f -> p o f", p=P))
        wo_sb = consts.tile([P, NHT, D], bf16, name="wo_sb")
        nc.gpsimd.dma_start(wo_sb[:], wo_ext.rearrange("(o p) e -> p o e", p=P))

        identity = consts.tile([P, P], bf16, name="identity")
        make_identity(nc, identity)
        ones_col = consts.tile([P, 1], bf16, name="ones_col")
        nc.any.memset(ones_col, 1.0)
        ones_row = consts.tile([1, P], bf16, name="ones_row")
        nc.any.memset(ones_row, 1.0)
        ones_row_f = consts.tile([1, P], f32, name="ones_row_f")
        nc.any.memset(ones_row_f, 1.0)
        bo_sb = consts.tile([1, D], f32, name="bo_sb")
        nc.sync.dma_start(bo_sb[:], bo_ext[None, :])
        jb_sb = consts.tile([P, NJT], f32, name="jb_sb")
        nc.sync.dma_start(jb_sb[:], jb_ext.rearrange("t p -> p t"))
        bo_bcast = consts.tile([P, D], f32, name="bo_bcast")
        with tc.tile_pool(name="initpsum", bufs=1, space="PSUM") as initp:
            bp0 = initp.tile([P, D], f32, name="bp0")
            nc.tensor.matmul(bp0[:], ones_row_f[:], bo_sb[:], start=True, stop=True)
            nc.any.tensor_copy(out=bo_bcast[:], in_=bp0[:])

        def load_xT(src_ext, r, tag):
            """DMA row r (cast f32->bf16) and PE-transpose to [d, pos] layout."""
            x_bf = xin_pool.tile([P, NPT, D], bf16, tag=f"xbf_{tag}")
            nc.gpsimd.dma_start(
                x_bf[:], src_ext[r].rearrange("(po pi) d -> pi po d", pi=P)
            )
            xT = xt_pool.tile([P, NDT, N], bf16, tag=f"xT_{tag}")
            for pt in range(NPT):
                for dh in range(NDT):
                    tp = tpsum.tile([P, P], bf16, tag="tp")
                    nc.tensor.transpose(
                        tp[:], x_bf[:, pt, dh * P : (dh + 1) * P], identity[:]
                    )
                    nc.any.tensor_copy(
                        out=xT[:, dh, pt * P : (pt + 1) * P], in_=tp[:]
                    )
            return xT

        # =====================  Phase 1: q/k + tied dots  =====================
        with ExitStack() as ph1:
            qk_pool = ph1.enter_context(tc.tile_pool(name="qk", bufs=1))
            dots_pool = ph1.enter_context(tc.tile_pool(name="dots", bufs=1))
            proj_psum = ph1.enter_context(
                tc.tile_pool(name="proj_psum", bufs=2, space="PSUM")
            )
            dots_psum = ph1.enter_context(
                tc.tile_pool(name="dots_psum", bufs=4, space="PSUM")
            )

            dots_sb = dots_pool.tile([P, H, NJT, N], f32, name="dots_sb")

            for gi, group in enumerate(groups):
                glen = len(group)
                q2 = qk_pool.tile([P, H, GMAX, N], bf16, tag="q2")
                k2 = qk_pool.tile([P, H, GMAX, N], bf16, tag="k2")
                for pq, pair in enumerate(group):
                    for parity in range(2):
                        r = 2 * pair + parity
                        xT = load_xT(x_ext, r, "p1")
                        if separate_xq:
                            xTq = load_xT(xq_ext, r, "p1q")
                        else:
                            xTq = xT
                        off = 64 * parity
                        for wsb, xtt, dest in ((wq_sb, xTq, q2), (wk_sb, xT, k2)):
                            for t in range(NHT):
                                pp = proj_psum.tile([P, N], f32, tag="pp")
                                for dt in range(NDT):
                                    nc.tensor.matmul(
                                        pp[:],
                                        wsb[:, dt, t * P : (t + 1) * P],
                                        xtt[:, dt, :],
                                        start=(dt == 0),
                                        stop=(dt == NDT - 1),
                                    )
                                nc.any.tensor_copy(
                                    out=dest[off : off + 64, 2 * t, pq, :],
                                    in_=pp[0:64, :],
                                )
                                nc.any.tensor_copy(
                                    out=dest[off : off + 64, 2 * t + 1, pq, :],
                                    in_=pp[64:128, :],
                                )
                # tied logits for this group: dotsT[h, j, i] += sum_pairs
                for h in range(H):
                    dps = [
                        dots_psum.tile([P, N], f32, tag="dp", name=f"dp{h}_{jt}")
                        for jt in range(NJT)
                    ]
                    # full 128-partition contraction sums over both rows of the
                    # pair at once (partitions = r_even.d | r_odd.d)
                    for pq in range(glen):
                        for jt in range(NJT):
                            nc.tensor.matmul(
                                dps[jt][:],
                                k2[:, h, pq, jt * P : (jt + 1) * P],
                                q2[:, h, pq, :],
                                start=(pq == 0),
                                stop=(pq == glen - 1),
                            )
                    for jt in range(NJT):
                        if gi == 0:
                            nc.any.tensor_copy(
                                out=dots_sb[:, h, jt, :], in_=dps[jt][:]
                            )
                        else:
                            nc.vector.tensor_add(
                                out=dots_sb[:, h, jt, :],
                                in0=dps[jt][:],
                                in1=dots_sb[:, h, jt, :],
                            )

            nc.sync.dma_start(cc_in[:], dots_sb[:])

        # =====================  AllReduce over the 8 cores  ===================
        nc.gpsimd.collective_compute(
            "AllReduce",
            ALU.add,
            replica_groups=[list(range(n_cores))],
            ins=[cc_in[:]],
            outs=[cc_out[:]],
        )

        # ============  Phase 2: v proj (overlaps AR), softmax, out  ===========
        with ExitStack() as ph2:
            exp_pool = ph2.enter_context(tc.tile_pool(name="expp", bufs=1))
            rs_pool = ph2.enter_context(tc.tile_pool(name="rsp", bufs=1))
            v2_pool = ph2.enter_context(tc.tile_pool(name="v2p", bufs=6))
            dl_pool = ph2.enter_context(tc.tile_pool(name="dlp", bufs=3))
            sm_pool = ph2.enter_context(tc.tile_pool(name="smp", bufs=2))
            out2_pool = ph2.enter_context(tc.tile_pool(name="o2p", bufs=2))
            yrow_pool = ph2.enter_context(tc.tile_pool(name="yrow", bufs=2))
            vpsum = ph2.enter_context(
                tc.tile_pool(name="vpsum", bufs=2, space="PSUM")
            )
            attpsum = ph2.enter_context(
                tc.tile_pool(name="attpsum", bufs=3, space="PSUM")
            )

            exp_sb = exp_pool.tile([P, H, NJT, N], bf16, name="exp_sb")
            rs_bcast = rs_pool.tile([P, H, N], bf16, name="rs_bcast")

            def emit_v(pair):
                """v projection for one row pair -> pair-stacked [j, (r0.hd|r1.hd)]."""
                v2 = v2_pool.tile([P, NJT, H, 2, DH], bf16, tag="v2", name=f"v2_{pair}")
                for parity in range(2):
                    r = 2 * pair + parity
                    xT = load_xT(x_ext, r, "p2")
                    for pt in range(NPT):
                        vp = vpsum.tile([P, INNER], f32, tag="vp")
                        for dt in range(NDT):
                            nc.tensor.matmul(
                                vp[:],
                                xT[:, dt, pt * P : (pt + 1) * P],
                                wv_sb[:, dt, :],
                                start=(dt == 0),
                                stop=(dt == NDT - 1),
                            )
                        nc.any.tensor_copy(
                            out=v2[:, pt, :, parity, :],
                            in_=vp.rearrange("p (h d) -> p h d", h=H),
                        )
                return v2

            # v projections for the first few pairs can overlap the AllReduce
            v2s = {}
            for pair in range(min(V_PREFETCH, PAIRS)):
                v2s[pair] = emit_v(pair)

            # ---- softmax on the reduced logits (replicated on every core) ----
            for h in range(H):
                dl = dl_pool.tile([P, NJT, N], f32, tag="dl")
                nc.sync.dma_start(dl[:], cc_out[:, h, :, :])
                for jt in range(NJT):
                    nc.scalar.activation(
                        exp_sb[:, h, jt, :],
                        dl[:, jt, :],
                        AF.Exp,
                        bias=jb_sb[:, jt : jt + 1],
                        scale=1.0,
                    )
                sp = vpsum.tile([1, N], f32, tag="vp")
                for jt in range(NJT):
                    nc.tensor.matmul(
                        sp[:],
                        ones_col[:],
                        exp_sb[:, h, jt, :],
                        start=(jt == 0),
                        stop=(jt == NJT - 1),
                    )
                s_sb = sm_pool.tile([1, N], f32, tag="s_sb")
                nc.any.tensor_copy(out=s_sb[:], in_=sp[:])
                rcp = sm_pool.tile([1, N], f32, tag="rcp")
                nc.vector.reciprocal(rcp[:], s_sb[:])
                rcp_bf = sm_pool.tile([1, N], bf16, tag="rcp_bf")
                nc.any.tensor_copy(out=rcp_bf[:], in_=rcp[:])
                bps = vpsum.tile([P, N], f32, tag="vp")
                nc.tensor.matmul(bps[:], ones_row[:], rcp_bf[:], start=True, stop=True)
                nc.any.tensor_copy(out=rs_bcast[:, h, :], in_=bps[:])

            # ---- group B: attn @ v, rescale, output projection, store ----
            for pair in range(PAIRS):
                if pair in v2s:
                    v2 = v2s[pair]
                else:
                    v2 = emit_v(pair)
                out2 = [
                    out2_pool.tile([P, NHT, N], bf16, tag="o2a", name=f"o2a_{pair}"),
                    out2_pool.tile([P, NHT, N], bf16, tag="o2b", name=f"o2b_{pair}"),
                ]
                for h in range(H):
                    ap_ps = attpsum.tile([P, N], f32, tag="att")
                    for jt in range(NJT):
                        nc.tensor.matmul(
                            ap_ps[:],
                            v2[:, jt, h, :, :],
                            exp_sb[:, h, jt, :],
                            start=(jt == 0),
                            stop=(jt == NJT - 1),
                        )
                    t, sub = h // 2, h % 2
                    for parity in range(2):
                        o = 64 * parity
                        nc.vector.tensor_tensor(
                            out2[parity][sub * 64 : sub * 64 + 64, t, :],
                            ap_ps[o : o + 64, :],
                            rs_bcast[o : o + 64, h, :],
                            ALU.mult,
                        )
                for parity in range(2):
                    r = 2 * pair + parity
                    yrow = yrow_pool.tile([P, NPT, D], f32, tag="yrow")
                    for it in range(NPT):
                        yp = attpsum.tile([P, D], f32, tag="att")
                        for t in range(NHT):
                            nc.tensor.matmul(
                                yp[:],
                                out2[parity][:, t, it * P : (it + 1) * P],
                                wo_sb[:, t, :],
                                start=(t == 0),
                                stop=(t == NHT - 1),
                            )
                        nc.vector.tensor_add(
                            out=yrow[:, it, :], in0=yp[:], in1=bo_bcast[:]
                        )
                    nc.sync.dma_start(
                        out_ext[r].rearrange("(po pi) e -> pi po e", pi=P), yrow[:]
                    )

    if do_finalize:
        nc.finalize()
    return nc


def _get_graph(separate_xq: bool, r_loc: int = R_LOC, n_cores: int = NCORES):
    key = (separate_xq, r_loc, n_cores)
    if key not in _graph_cache:
        _graph_cache[key] = _build(separate_xq, r_loc, n_cores)
    return _graph_cache[key]


def _prepare(x, mask, Wq, Wk, Wv, Wo, bo, tie_attn_dim):
    """Host-side prep: mask bookkeeping, weight folding, sharded in_maps."""
    x = np.ascontiguousarray(np.asarray(x, dtype=np.float32))
    mask = np.asarray(mask).astype(bool)
    Wq = np.asarray(Wq, dtype=np.float32)
    Wk = np.ascontiguousarray(np.asarray(Wk, dtype=np.float32))
    Wv = np.ascontiguousarray(np.asarray(Wv, dtype=np.float32))
    Wo = np.ascontiguousarray(np.asarray(Wo, dtype=np.float32))
    bo = np.ascontiguousarray(np.asarray(bo, dtype=np.float32))
    r = int(tie_attn_dim)
    assert x.shape == (B * R, N, D) and r == R, (x.shape, r)

    m = mask.reshape(B, R, N)
    has_rows = m.any(axis=-1)[0]  # [R]
    num_rows = max(int(has_rows.sum()), 1)
    col_valid = m.any(axis=1)[0]  # [N]

    scale = (DH ** -0.5) * (num_rows ** -0.5)
    Wq_eff = np.ascontiguousarray(Wq * np.float32(scale))

    jbias = np.where(col_valid, 0.0, -1e30).astype(np.float32)
    jbias = np.ascontiguousarray(jbias.reshape(NJT, P))

    separate_xq = not bool(has_rows.all())
    if separate_xq:
        xq = np.ascontiguousarray(x * has_rows[:, None, None].astype(np.float32))
    else:
        xq = None

    in_maps = []
    for c in range(NCORES):
        im = {
            "x": np.ascontiguousarray(x[c * R_LOC : (c + 1) * R_LOC]),
            "Wq": Wq_eff,
            "Wk": Wk,
            "Wv": Wv,
            "Wo": Wo,
            "bo": bo,
            "jbias": jbias,
        }
        if separate_xq:
            im["xq"] = np.ascontiguousarray(xq[c * R_LOC : (c + 1) * R_LOC])
        in_maps.append(im)
    return separate_xq, in_maps


def kernel(x, mask, Wq, Wk, Wv, Wo, bo, tie_attn_dim):
    from concourse.bass_utils import run_bass_kernel_spmd

    separate_xq, in_maps = _prepare(x, mask, Wq, Wk, Wv, Wo, bo, tie_attn_dim)
    nc = _get_graph(separate_xq)
    res = run_bass_kernel_spmd(nc, in_maps, list(range(NCORES)))
    out = np.concatenate([res.results[c]["out"] for c in range(NCORES)], axis=0)
    return out.astype(np.float32)


def _install_ntff_hook():
    """The agent image's antenv lacks axon_hooks; recreate it so trace=True
    can drive NTFF profiling through libaxon_pjrt.so (see trn_boot.py)."""
    try:
        from antenv import axon_hooks  # noqa: F401

        return
    except ImportError:
        pass
    import types

    import antenv

    mod = types.ModuleType("antenv.axon_hooks")
    holder = {}
    mod.set_axon_ntff_profile_hook = lambda h: holder.__setitem__("h", h)
    mod.get_axon_ntff_profile_hook = lambda: holder.get("h")
    sys.modules["antenv.axon_hooks"] = mod
    antenv.axon_hooks = mod
    if "/root/.axon_site" not in sys.path:
        sys.path.insert(0, "/root/.axon_site")
    from trn_agent_boot.trn_boot import _ntff_profile_via_ctypes

    mod.set_axon_ntff_profile_hook(
        _ntff_profile_via_ctypes("/opt/axon/libaxon_pjrt.so")
    )


def bench(inputs):
    """Run with neuron-profile tracing; returns (exec_time_ns, output)."""
    from concourse.bass_utils import run_bass_kernel_spmd

    _install_ntff_hook()

    separate_xq, in_maps = _prepare(**inputs)
    nc = _get_graph(separate_xq)
    res = run_bass_kernel_spmd(nc, in_maps, list(range(NCORES)), trace=True)
    out = np.concatenate([res.results[c]["out"] for c in range(NCORES)], axis=0)
    return res, out.astype(np.float32)


# revision 22
# speedup vs baseline: 1.0106x; 1.0106x over previous
"""Trainium2 Bass kernel for tied-row MSA attention (nn_Attention_52329881535135).

Strategy (8 NeuronCores, one chip):
  - Shard the MSA row dim r (leading b*r=256) across the 8 cores: 32 rows each.
  - Each core: q/k projections for its rows into transposed per-head layout,
    accumulate the row-tied logits dots[h,j,i] = sum_r k_r^T q_r in PSUM/SBUF.
  - AllReduce the 8.4MB fp32 logits across the 8 cores (sum over all 256 rows).
  - Every core computes the (replicated) softmax, then out = attn @ v for its
    own 32 rows, followed by the output projection; host concatenates shards.

  Mask bookkeeping (has_rows / num_rows / mask_any) is computed on the host at
  call time and folded into the weights / an additive column bias, so the
  device graph only does dense matmuls.

  Compute dtype: bf16 matmul inputs, fp32 PSUM accumulation, fp32 logits
  and AllReduce, fp32 softmax normalization.
"""

import sys

sys.path.insert(0, "/opt/trn_rl_repo")

import numpy as np

B, R, N, D, H, DH = 1, 256, 512, 256, 8, 64
INNER = H * DH
NCORES = 8
R_LOC = R // NCORES  # 32 rows per core
P = 128
NPT = N // P  # 4 position tiles
NJT = N // P  # 4 j tiles
NDT = D // P  # 2 d tiles
NHT = INNER // P  # 4 hd tiles
V_PREFETCH = 8  # pairs whose v-projection is emitted before softmax (AR overlap)

_graph_cache = {}
_result_cache = {}


def _build(
    separate_xq: bool,
    r_loc: int = R_LOC,
    n_cores: int = NCORES,
    do_finalize: bool = True,
):
    from contextlib import ExitStack

    from concourse import bacc, mybir, tile
    from concourse.masks import make_identity

    f32 = mybir.dt.float32
    bf16 = mybir.dt.bfloat16
    AF = mybir.ActivationFunctionType
    ALU = mybir.AluOpType

    nc = bacc.Bacc(
        "TRN2", target_bir_lowering=False, debug=False, num_devices=n_cores
    )

    x_ext = nc.declare_dram_parameter("x", [r_loc, N, D], f32, isOutput=False)
    if separate_xq:
        xq_ext = nc.declare_dram_parameter("xq", [r_loc, N, D], f32, isOutput=False)
    else:
        xq_ext = x_ext
    wq_ext = nc.declare_dram_parameter("Wq", [D, INNER], f32, isOutput=False)
    wk_ext = nc.declare_dram_parameter("Wk", [D, INNER], f32, isOutput=False)
    wv_ext = nc.declare_dram_parameter("Wv", [D, INNER], f32, isOutput=False)
    wo_ext = nc.declare_dram_parameter("Wo", [INNER, D], f32, isOutput=False)
    bo_ext = nc.declare_dram_parameter("bo", [D], f32, isOutput=False)
    jb_ext = nc.declare_dram_parameter("jbias", [NJT, P], f32, isOutput=False)
    out_ext = nc.declare_dram_parameter("out", [r_loc, N, D], f32, isOutput=True)

    cc_shape = [P, H, NJT, N]
    cc_in = nc.dram_tensor("cc_in", cc_shape, f32)
    cc_out = nc.dram_tensor(
        "cc_out", cc_shape, f32, addr_space="Shared" if n_cores > 4 else "Local"
    )
    xbf_dram = nc.dram_tensor("xbf_dram", [r_loc, N, D], bf16)
    if separate_xq:
        xqbf_dram = nc.dram_tensor("xqbf_dram", [r_loc, N, D], bf16)

    PAIRS = r_loc // 2
    # pair groups of <=4 pairs (8 rows): bounds SBUF held q/k projections
    groups = [list(range(g, min(g + 4, PAIRS))) for g in range(0, PAIRS, 4)]
    GMAX = max(len(g) for g in groups)

    with tile.TileContext(nc) as tc, ExitStack() as top:
        consts = top.enter_context(tc.tile_pool(name="consts", bufs=1))
        xt_pool = top.enter_context(tc.tile_pool(name="xt", bufs=3))

        # --- constants / weights (resident in SBUF, cast to bf16 on load) ---
        wq_sb = consts.tile([P, NDT, INNER], bf16, name="wq_sb")
        nc.gpsimd.dma_start(wq_sb[:], wq_ext.rearrange("(o p) f -> p o f", p=P))
        wk_sb = consts.tile([P, NDT, INNER], bf16, name="wk_sb")
        nc.gpsimd.dma_start(wk_sb[:], wk_ext.rearrange("(o p) f -> p o f", p=P))
        wv_sb = consts.tile([P, NDT, INNER], bf16, name="wv_sb")
        nc.gpsimd.dma_start(wv_sb[:], wv_ext.rearrange("(o p) f -> p o f", p=P))
        wo_sb = consts.tile([P, NHT, D], bf16, name="wo_sb")
        nc.gpsimd.dma_start(wo_sb[:], wo_ext.rearrange("(o p) e -> p o e", p=P))

        ones_col = consts.tile([P, 1], bf16, name="ones_col")
        nc.any.memset(ones_col, 1.0)
        ones_row = consts.tile([1, P], bf16, name="ones_row")
        nc.any.memset(ones_row, 1.0)
        ones_row_f = consts.tile([1, P], f32, name="ones_row_f")
        nc.any.memset(ones_row_f, 1.0)
        bo_sb = consts.tile([1, D], f32, name="bo_sb")
        nc.sync.dma_start(bo_sb[:], bo_ext[None, :])
        jb_sb = consts.tile([P, NJT], f32, name="jb_sb")
        nc.sync.dma_start(jb_sb[:], jb_ext.rearrange("t p -> p t"))
        bo_bcast = consts.tile([P, D], f32, name="bo_bcast")
        with tc.tile_pool(name="initpsum", bufs=1, space="PSUM") as initp:
            bp0 = initp.tile([P, D], f32, name="bp0")
            nc.tensor.matmul(bp0[:], ones_row_f[:], bo_sb[:], start=True, stop=True)
            nc.any.tensor_copy(out=bo_bcast[:], in_=bp0[:])

        def cast_x(src_ext, dst_dram, r):
            """DRAM->DRAM cast f32->bf16 of row r (SWDGE cast DMA)."""
            nc.gpsimd.dma_start(dst_dram[r], src_ext[r])

        def load_xT(src_dram, r, tag):
            """DMA-transpose row r of the bf16 staging tensor to [d, pos]."""
            xT = xt_pool.tile([P, NDT, N], bf16, tag=f"xT_{tag}")
            for dh in range(NDT):
                nc.sync.dma_start_transpose(
                    xT[:, dh, :], src_dram[r][:, dh * P : (dh + 1) * P]
                )
            return xT

        # =====================  Phase 1: q/k + tied dots  =====================
        with ExitStack() as ph1:
            qk_pool = ph1.enter_context(tc.tile_pool(name="qk", bufs=1))
            dots_pool = ph1.enter_context(tc.tile_pool(name="dots", bufs=1))
            proj_psum = ph1.enter_context(
                tc.tile_pool(name="proj_psum", bufs=3, space="PSUM")
            )
            dots_psum = ph1.enter_context(
                tc.tile_pool(name="dots_psum", bufs=4, space="PSUM")
            )

            dots_sb = dots_pool.tile([P, H, NJT, N], f32, name="dots_sb")

            for gi, group in enumerate(groups):
                glen = len(group)
                q2 = qk_pool.tile([P, H, GMAX, N], bf16, tag="q2")
                k2 = qk_pool.tile([P, H, GMAX, N], bf16, tag="k2")
                for pq, pair in enumerate(group):
                    for parity in range(2):
                        r = 2 * pair + parity
                        cast_x(x_ext, xbf_dram, r)
                        xT = load_xT(xbf_dram, r, "p1")
                        if separate_xq:
                            cast_x(xq_ext, xqbf_dram, r)
                            xTq = load_xT(xqbf_dram, r, "p1q")
                        else:
                            xTq = xT
                        off = 64 * parity
                        for wsb, xtt, dest, evac in (
                            (wq_sb, xTq, q2, nc.vector.tensor_copy),
                            (wk_sb, xT, k2, nc.scalar.copy),
                        ):
                            for t in range(NHT):
                                pp = proj_psum.tile([P, N], f32, tag="pp")
                                for dt in range(NDT):
                                    nc.tensor.matmul(
                                        pp[:],
                                        wsb[:, dt, t * P : (t + 1) * P],
                                        xtt[:, dt, :],
                                        start=(dt == 0),
                                        stop=(dt == NDT - 1),
                                    )
                                evac(dest[off : off + 64, 2 * t, pq, :], pp[0:64, :])
                                evac(
                                    dest[off : off + 64, 2 * t + 1, pq, :],
                                    pp[64:128, :],
                                )
                # tied logits for this group: dotsT[h, j, i] += sum_pairs
                for h in range(H):
                    dps = [
                        dots_psum.tile([P, N], f32, tag="dp", name=f"dp{h}_{jt}")
                        for jt in range(NJT)
                    ]
                    # full 128-partition contraction sums over both rows of the
                    # pair at once (partitions = r_even.d | r_odd.d)
                    for pq in range(glen):
                        for jt in range(NJT):
                            nc.tensor.matmul(
                                dps[jt][:],
                                k2[:, h, pq, jt * P : (jt + 1) * P],
                                q2[:, h, pq, :],
                                start=(pq == 0),
                                stop=(pq == glen - 1),
                            )
                    for jt in range(NJT):
                        if gi == 0:
                            nc.vector.tensor_copy(dots_sb[:, h, jt, :], dps[jt][:])
                        else:
                            nc.vector.tensor_add(
                                out=dots_sb[:, h, jt, :],
                                in0=dps[jt][:],
                                in1=dots_sb[:, h, jt, :],
                            )

            nc.sync.dma_start(cc_in[:], dots_sb[:])

        # =====================  AllReduce over the 8 cores  ===================
        nc.gpsimd.collective_compute(
            "AllReduce",
            ALU.add,
            replica_groups=[list(range(n_cores))],
            ins=[cc_in[:]],
            outs=[cc_out[:]],
        )

        # ============  Phase 2: v proj (overlaps AR), softmax, out  ===========
        with ExitStack() as ph2:
            exp_pool = ph2.enter_context(tc.tile_pool(name="expp", bufs=1))
            rs_pool = ph2.enter_context(tc.tile_pool(name="rsp", bufs=1))
            v2_pool = ph2.enter_context(tc.tile_pool(name="v2p", bufs=8))
            dl_pool = ph2.enter_context(tc.tile_pool(name="dlp", bufs=3))
            sm_pool = ph2.enter_context(tc.tile_pool(name="smp", bufs=2))
            out2_pool = ph2.enter_context(tc.tile_pool(name="o2p", bufs=2))
            yrow_pool = ph2.enter_context(tc.tile_pool(name="yrow", bufs=2))
            vpsum = ph2.enter_context(
                tc.tile_pool(name="vpsum", bufs=3, space="PSUM")
            )
            attpsum = ph2.enter_context(
                tc.tile_pool(name="attpsum", bufs=4, space="PSUM")
            )

            exp_sb = exp_pool.tile([P, H, NJT, N], bf16, name="exp_sb")
            rs_bcast = rs_pool.tile([P, H, N], bf16, name="rs_bcast")

            def emit_v(pair):
                """v projection for one row pair -> pair-stacked [j, (r0.hd|r1.hd)]."""
                v2 = v2_pool.tile([P, NJT, H, 2, DH], bf16, tag="v2", name=f"v2_{pair}")
                for parity in range(2):
                    r = 2 * pair + parity
                    xT = load_xT(xbf_dram, r, "p2")
                    for pt in range(NPT):
                        vp = vpsum.tile([P, INNER], f32, tag="vp")
                        for dt in range(NDT):
                            nc.tensor.matmul(
                                vp[:],
                                xT[:, dt, pt * P : (pt + 1) * P],
                                wv_sb[:, dt, :],
                                start=(dt == 0),
                                stop=(dt == NDT - 1),
                            )
                        nc.scalar.copy(
                            v2[:, pt, :, parity, :],
                            vp.rearrange("p (h d) -> p h d", h=H),
                        )
                return v2

            # v projections for the first few pairs can overlap the AllReduce
            v2s = {}
            for pair in range(min(V_PREFETCH, PAIRS)):
                v2s[pair] = emit_v(pair)

            # ---- softmax on the reduced logits (replicated on every core) ----
            for h in range(H):
                dl = dl_pool.tile([P, NJT, N], f32, tag="dl")
                nc.sync.dma_start(dl[:], cc_out[:, h, :, :])
                for jt in range(NJT):
                    nc.scalar.activation(
                        exp_sb[:, h, jt, :],
                        dl[:, jt, :],
                        AF.Exp,
                        bias=jb_sb[:, jt : jt + 1],
                        scale=1.0,
                    )
                sp = vpsum.tile([1, N], f32, tag="vp")
                for jt in range(NJT):
                    nc.tensor.matmul(
                        sp[:],
                        ones_col[:],
                        exp_sb[:, h, jt, :],
                        start=(jt == 0),
                        stop=(jt == NJT - 1),
                    )
                s_sb = sm_pool.tile([1, N], f32, tag="s_sb")
                nc.any.tensor_copy(out=s_sb[:], in_=sp[:])
                rcp = sm_pool.tile([1, N], f32, tag="rcp")
                nc.vector.reciprocal(rcp[:], s_sb[:])
                rcp_bf = sm_pool.tile([1, N], bf16, tag="rcp_bf")
                nc.any.tensor_copy(out=rcp_bf[:], in_=rcp[:])
                bps = vpsum.tile([P, N], f32, tag="vp")
                nc.tensor.matmul(bps[:], ones_row[:], rcp_bf[:], start=True, stop=True)
                nc.any.tensor_copy(out=rs_bcast[:, h, :], in_=bps[:])

            # ---- group B: attn @ v, rescale, output projection, store ----
            for pair in range(PAIRS):
                if pair in v2s:
                    v2 = v2s[pair]
                else:
                    v2 = emit_v(pair)
                out2 = [
                    out2_pool.tile([P, NHT, N], bf16, tag="o2a", name=f"o2a_{pair}"),
                    out2_pool.tile([P, NHT, N], bf16, tag="o2b", name=f"o2b_{pair}"),
                ]
                for h in range(H):
                    ap_ps = attpsum.tile([P, N], f32, tag="att")
                    for jt in range(NJT):
                        nc.tensor.matmul(
                            ap_ps[:],
                            v2[:, jt, h, :, :],
                            exp_sb[:, h, jt, :],
                            start=(jt == 0),
                            stop=(jt == NJT - 1),
                        )
                    t, sub = h // 2, h % 2
                    for parity in range(2):
                        o = 64 * parity
                        nc.vector.tensor_tensor(
                            out2[parity][sub * 64 : sub * 64 + 64, t, :],
                            ap_ps[o : o + 64, :],
                            rs_bcast[o : o + 64, h, :],
                            ALU.mult,
                        )
                for parity in range(2):
                    r = 2 * pair + parity
                    yrow = yrow_pool.tile([P, NPT, D], f32, tag="yrow")
                    for it in range(NPT):
                        yp = attpsum.tile([P, D], f32, tag="att")
                        for t in range(NHT):
                            nc.tensor.matmul(
                                yp[:],
                                out2[parity][:, t, it * P : (it + 1) * P],
                                wo_sb[:, t, :],
                                start=(t == 0),
                                stop=(t == NHT - 1),
                            )
                        nc.vector.tensor_add(
                            out=yrow[:, it, :], in0=yp[:], in1=bo_bcast[:]
                        )
                    nc.sync.dma_start(
                        out_ext[r].rearrange("(po pi) e -> pi po e", pi=P), yrow[:]
                    )

    if do_finalize:
        nc.finalize()
    return nc


def _get_graph(separate_xq: bool, r_loc: int = R_LOC, n_cores: int = NCORES):
    key = (separate_xq, r_loc, n_cores)
    if key not in _graph_cache:
        _graph_cache[key] = _build(separate_xq, r_loc, n_cores)
    return _graph_cache[key]


def _prepare(x, mask, Wq, Wk, Wv, Wo, bo, tie_attn_dim):
    """Host-side prep: mask bookkeeping, weight folding, sharded in_maps."""
    x = np.ascontiguousarray(np.asarray(x, dtype=np.float32))
    mask = np.asarray(mask).astype(bool)
    Wq = np.asarray(Wq, dtype=np.float32)
    Wk = np.ascontiguousarray(np.asarray(Wk, dtype=np.float32))
    Wv = np.ascontiguousarray(np.asarray(Wv, dtype=np.float32))
    Wo = np.ascontiguousarray(np.asarray(Wo, dtype=np.float32))
    bo = np.ascontiguousarray(np.asarray(bo, dtype=np.float32))
    r = int(tie_attn_dim)
    assert x.shape == (B * R, N, D) and r == R, (x.shape, r)

    m = mask.reshape(B, R, N)
    has_rows = m.any(axis=-1)[0]  # [R]
    num_rows = max(int(has_rows.sum()), 1)
    col_valid = m.any(axis=1)[0]  # [N]

    scale = (DH ** -0.5) * (num_rows ** -0.5)
    Wq_eff = np.ascontiguousarray(Wq * np.float32(scale))

    jbias = np.where(col_valid, 0.0, -1e30).astype(np.float32)
    jbias = np.ascontiguousarray(jbias.reshape(NJT, P))

    separate_xq = not bool(has_rows.all())
    if separate_xq:
        xq = np.ascontiguousarray(x * has_rows[:, None, None].astype(np.float32))
    else:
        xq = None

    in_maps = []
    for c in range(NCORES):
        im = {
            "x": np.ascontiguousarray(x[c * R_LOC : (c + 1) * R_LOC]),
            "Wq": Wq_eff,
            "Wk": Wk,
            "Wv": Wv,
            "Wo": Wo,
            "bo": bo,
            "jbias": jbias,
        }
        if separate_xq:
            im["xq"] = np.ascontiguousarray(xq[c * R_LOC : (c + 1) * R_LOC])
        in_maps.append(im)
    return separate_xq, in_maps


def kernel(x, mask, Wq, Wk, Wv, Wo, bo, tie_attn_dim):
    from concourse.bass_utils import run_bass_kernel_spmd

    separate_xq, in_maps = _prepare(x, mask, Wq, Wk, Wv, Wo, bo, tie_attn_dim)
    nc = _get_graph(separate_xq)
    res = run_bass_kernel_spmd(nc, in_maps, list(range(NCORES)))
    out = np.concatenate([res.results[c]["out"] for c in range(NCORES)], axis=0)
    return out.astype(np.float32)


def _install_ntff_hook():
    """The agent image's antenv lacks axon_hooks; recreate it so trace=True
    can drive NTFF profiling through libaxon_pjrt.so (see trn_boot.py)."""
    try:
        from antenv import axon_hooks  # noqa: F401

        return
    except ImportError:
        pass
    import types

    import antenv

    mod = types.ModuleType("antenv.axon_hooks")
    holder = {}
    mod.set_axon_ntff_profile_hook = lambda h: holder.__setitem__("h", h)
    mod.get_axon_ntff_profile_hook = lambda: holder.get("h")
    sys.modules["antenv.axon_hooks"] = mod
    antenv.axon_hooks = mod
    if "/root/.axon_site" not in sys.path:
        sys.path.insert(0, "/root/.axon_site")
    from trn_agent_boot.trn_boot import _ntff_profile_via_ctypes

    mod.set_axon_ntff_profile_hook(
        _ntff_profile_via_ctypes("/opt/axon/libaxon_pjrt.so")
    )


def bench(inputs):
    """Run with neuron-profile tracing; returns (exec_time_ns, output)."""
    from concourse.bass_utils import run_bass_kernel_spmd

    _install_ntff_hook()

    separate_xq, in_maps = _prepare(**inputs)
    nc = _get_graph(separate_xq)
    res = run_bass_kernel_spmd(nc, in_maps, list(range(NCORES)), trace=True)
    out = np.concatenate([res.results[c]["out"] for c in range(NCORES)], axis=0)
    return res, out.astype(np.float32)


# revision 29
# speedup vs baseline: 1.0229x; 1.0122x over previous
"""Trainium2 Bass kernel for tied-row MSA attention (nn_Attention_52329881535135).

Strategy (8 NeuronCores, one chip):
  - Shard the MSA row dim r (leading b*r=256) across the 8 cores: 32 rows each.
  - Each core: q/k projections for its rows into transposed per-head layout,
    accumulate the row-tied logits dots[h,j,i] = sum_r k_r^T q_r in PSUM/SBUF.
  - AllReduce the 8.4MB fp32 logits across the 8 cores (sum over all 256 rows).
  - Every core computes the (replicated) softmax, then out = attn @ v for its
    own 32 rows, followed by the output projection; host concatenates shards.

  Mask bookkeeping (has_rows / num_rows / mask_any) is computed on the host at
  call time and folded into the weights / an additive column bias, so the
  device graph only does dense matmuls.

  Compute dtype: bf16 matmul inputs, fp32 PSUM accumulation, fp32 logits
  and AllReduce, fp32 softmax normalization.
"""

import sys

sys.path.insert(0, "/opt/trn_rl_repo")

import numpy as np

B, R, N, D, H, DH = 1, 256, 512, 256, 8, 64
INNER = H * DH
NCORES = 8
R_LOC = R // NCORES  # 32 rows per core
P = 128
NPT = N // P  # 4 position tiles
NJT = N // P  # 4 j tiles
NDT = D // P  # 2 d tiles
NHT = INNER // P  # 4 hd tiles
V_PREFETCH = 8  # pairs whose v-projection is emitted before softmax (AR overlap)

_graph_cache = {}
_result_cache = {}


def _build(
    separate_xq: bool,
    has_bias: bool = True,
    r_loc: int = R_LOC,
    n_cores: int = NCORES,
    do_finalize: bool = True,
):
    from contextlib import ExitStack

    from concourse import bacc, mybir, tile
    from concourse.masks import make_identity

    f32 = mybir.dt.float32
    bf16 = mybir.dt.bfloat16
    AF = mybir.ActivationFunctionType
    ALU = mybir.AluOpType

    nc = bacc.Bacc(
        "TRN2", target_bir_lowering=False, debug=False, num_devices=n_cores
    )

    x_ext = nc.declare_dram_parameter("x", [r_loc, N, D], f32, isOutput=False)
    if separate_xq:
        xq_ext = nc.declare_dram_parameter("xq", [r_loc, N, D], f32, isOutput=False)
    else:
        xq_ext = x_ext
    wq_ext = nc.declare_dram_parameter("Wq", [D, INNER], f32, isOutput=False)
    wk_ext = nc.declare_dram_parameter("Wk", [D, INNER], f32, isOutput=False)
    wv_ext = nc.declare_dram_parameter("Wv", [D, INNER], f32, isOutput=False)
    wo_ext = nc.declare_dram_parameter("Wo", [INNER, D], f32, isOutput=False)
    bo_ext = nc.declare_dram_parameter("bo", [D], f32, isOutput=False)
    jb_ext = nc.declare_dram_parameter("jbias", [NJT, P], f32, isOutput=False)
    out_ext = nc.declare_dram_parameter("out", [r_loc, N, D], f32, isOutput=True)

    # logits AllReduce split in two (heads 0..3 / 4..7) so the first collective
    # overlaps the tail of phase 1 and the second overlaps softmax+attn of the
    # first head group
    HH = H // 2
    cc_shape = [P, HH, NJT, N]
    out_space = "Shared" if n_cores > 4 else "Local"
    cc_in_a = nc.dram_tensor("cc_in_a", cc_shape, f32)
    cc_out_a = nc.dram_tensor("cc_out_a", cc_shape, f32, addr_space=out_space)
    cc_in_b = nc.dram_tensor("cc_in_b", cc_shape, f32)
    cc_out_b = nc.dram_tensor("cc_out_b", cc_shape, f32, addr_space=out_space)
    xbf_dram = nc.dram_tensor("xbf_dram", [r_loc, N, D], bf16)
    if separate_xq:
        xqbf_dram = nc.dram_tensor("xqbf_dram", [r_loc, N, D], bf16)

    PAIRS = r_loc // 2
    # pair groups of <=4 pairs (8 rows): bounds SBUF held q/k projections
    groups = [list(range(g, min(g + 4, PAIRS))) for g in range(0, PAIRS, 4)]
    GMAX = max(len(g) for g in groups)

    with tile.TileContext(nc) as tc, ExitStack() as top:
        consts = top.enter_context(tc.tile_pool(name="consts", bufs=1))
        xt_pool = top.enter_context(tc.tile_pool(name="xt", bufs=3))

        # --- constants / weights (resident in SBUF, cast to bf16 on load) ---
        wq_sb = consts.tile([P, NDT, INNER], bf16, name="wq_sb")
        nc.gpsimd.dma_start(wq_sb[:], wq_ext.rearrange("(o p) f -> p o f", p=P))
        wk_sb = consts.tile([P, NDT, INNER], bf16, name="wk_sb")
        nc.gpsimd.dma_start(wk_sb[:], wk_ext.rearrange("(o p) f -> p o f", p=P))
        wv_sb = consts.tile([P, NDT, INNER], bf16, name="wv_sb")
        nc.gpsimd.dma_start(wv_sb[:], wv_ext.rearrange("(o p) f -> p o f", p=P))
        wo_sb = consts.tile([P, NHT, D], bf16, name="wo_sb")
        nc.gpsimd.dma_start(wo_sb[:], wo_ext.rearrange("(o p) e -> p o e", p=P))

        ones_col = consts.tile([P, 1], bf16, name="ones_col")
        nc.any.memset(ones_col, 1.0)
        ones_row = consts.tile([1, P], bf16, name="ones_row")
        nc.any.memset(ones_row, 1.0)
        ones_row_f = consts.tile([1, P], f32, name="ones_row_f")
        nc.any.memset(ones_row_f, 1.0)
        bo_sb = consts.tile([1, D], f32, name="bo_sb")
        nc.sync.dma_start(bo_sb[:], bo_ext[None, :])
        jb_sb = consts.tile([P, NJT], f32, name="jb_sb")
        nc.sync.dma_start(jb_sb[:], jb_ext.rearrange("t p -> p t"))
        if has_bias:
            bo_bcast = consts.tile([P, D], f32, name="bo_bcast")
            with tc.tile_pool(name="initpsum", bufs=1, space="PSUM") as initp:
                bp0 = initp.tile([P, D], f32, name="bp0")
                nc.tensor.matmul(
                    bp0[:], ones_row_f[:], bo_sb[:], start=True, stop=True
                )
                nc.any.tensor_copy(out=bo_bcast[:], in_=bp0[:])

        def cast_x(src_ext, dst_dram, r):
            """DRAM->DRAM cast f32->bf16 of row r (SWDGE cast DMA)."""
            nc.gpsimd.dma_start(dst_dram[r], src_ext[r])

        def load_xT(src_dram, r, tag):
            """DMA-transpose row r of the bf16 staging tensor to [d, pos]."""
            xT = xt_pool.tile([P, NDT, N], bf16, tag=f"xT_{tag}")
            for dh in range(NDT):
                nc.sync.dma_start_transpose(
                    xT[:, dh, :], src_dram[r][:, dh * P : (dh + 1) * P]
                )
            return xT

        # =====================  Phase 1: q/k + tied dots  =====================
        with ExitStack() as ph1:
            qk_pool = ph1.enter_context(tc.tile_pool(name="qk", bufs=1))
            dots_pool = ph1.enter_context(tc.tile_pool(name="dots", bufs=1))
            proj_psum = ph1.enter_context(
                tc.tile_pool(name="proj_psum", bufs=3, space="PSUM")
            )
            dots_psum = ph1.enter_context(
                tc.tile_pool(name="dots_psum", bufs=4, space="PSUM")
            )

            dots_sb = dots_pool.tile([P, H, NJT, N], f32, name="dots_sb")

            for gi, group in enumerate(groups):
                glen = len(group)
                q2 = qk_pool.tile([P, H, GMAX, N], bf16, tag="q2")
                k2 = qk_pool.tile([P, H, GMAX, N], bf16, tag="k2")
                for pq, pair in enumerate(group):
                    for parity in range(2):
                        r = 2 * pair + parity
                        cast_x(x_ext, xbf_dram, r)
                        xT = load_xT(xbf_dram, r, "p1")
                        if separate_xq:
                            cast_x(xq_ext, xqbf_dram, r)
                            xTq = load_xT(xqbf_dram, r, "p1q")
                        else:
                            xTq = xT
                        off = 64 * parity
                        for wsb, xtt, dest, evac in (
                            (wq_sb, xTq, q2, nc.vector.tensor_copy),
                            (wk_sb, xT, k2, nc.scalar.copy),
                        ):
                            for t in range(NHT):
                                pp = proj_psum.tile([P, N], f32, tag="pp")
                                for dt in range(NDT):
                                    nc.tensor.matmul(
                                        pp[:],
                                        wsb[:, dt, t * P : (t + 1) * P],
                                        xtt[:, dt, :],
                                        start=(dt == 0),
                                        stop=(dt == NDT - 1),
                                    )
                                evac(dest[off : off + 64, 2 * t, pq, :], pp[0:64, :])
                                evac(
                                    dest[off : off + 64, 2 * t + 1, pq, :],
                                    pp[64:128, :],
                                )
                # tied logits for this group: dotsT[h, j, i] += sum_pairs
                for h in range(H):
                    dps = [
                        dots_psum.tile([P, N], f32, tag="dp", name=f"dp{h}_{jt}")
                        for jt in range(NJT)
                    ]
                    # full 128-partition contraction sums over both rows of the
                    # pair at once (partitions = r_even.d | r_odd.d)
                    for pq in range(glen):
                        for jt in range(NJT):
                            nc.tensor.matmul(
                                dps[jt][:],
                                k2[:, h, pq, jt * P : (jt + 1) * P],
                                q2[:, h, pq, :],
                                start=(pq == 0),
                                stop=(pq == glen - 1),
                            )
                    for jt in range(NJT):
                        if gi == 0:
                            nc.vector.tensor_copy(dots_sb[:, h, jt, :], dps[jt][:])
                        else:
                            nc.vector.tensor_add(
                                out=dots_sb[:, h, jt, :],
                                in0=dps[jt][:],
                                in1=dots_sb[:, h, jt, :],
                            )
                    if gi == len(groups) - 1 and h in (HH - 1, H - 1):
                        cin = cc_in_a if h < HH else cc_in_b
                        cout = cc_out_a if h < HH else cc_out_b
                        hsl = slice(0, HH) if h < HH else slice(HH, H)
                        nc.sync.dma_start(cin[:], dots_sb[:, hsl, :, :])
                        nc.gpsimd.collective_compute(
                            "AllReduce",
                            ALU.add,
                            replica_groups=[list(range(n_cores))],
                            ins=[cin[:]],
                            outs=[cout[:]],
                        )

        # ============  Phase 2: v proj (overlaps AR), softmax, out  ===========
        with ExitStack() as ph2:
            exp_pool = ph2.enter_context(tc.tile_pool(name="expp", bufs=1))
            rs_pool = ph2.enter_context(tc.tile_pool(name="rsp", bufs=1))
            v2_pool = ph2.enter_context(tc.tile_pool(name="v2p", bufs=8))
            dl_pool = ph2.enter_context(tc.tile_pool(name="dlp", bufs=2))
            sm_pool = ph2.enter_context(tc.tile_pool(name="smp", bufs=2))
            out2_pool = ph2.enter_context(tc.tile_pool(name="o2p", bufs=6))
            yrow_pool = ph2.enter_context(tc.tile_pool(name="yrow", bufs=2))
            vpsum = ph2.enter_context(
                tc.tile_pool(name="vpsum", bufs=3, space="PSUM")
            )
            attpsum = ph2.enter_context(
                tc.tile_pool(name="attpsum", bufs=4, space="PSUM")
            )

            exp_sb = exp_pool.tile([P, H, NJT, N], bf16, name="exp_sb")
            rs_bcast = rs_pool.tile([P, H, N], bf16, name="rs_bcast")

            def emit_v(pair):
                """v projection for one row pair -> pair-stacked [j, (r0.hd|r1.hd)]."""
                v2 = v2_pool.tile([P, NJT, H, 2, DH], bf16, tag="v2", name=f"v2_{pair}")
                for parity in range(2):
                    r = 2 * pair + parity
                    xT = load_xT(xbf_dram, r, "p2")
                    for pt in range(NPT):
                        vp = vpsum.tile([P, INNER], f32, tag="vp")
                        for dt in range(NDT):
                            nc.tensor.matmul(
                                vp[:],
                                xT[:, dt, pt * P : (pt + 1) * P],
                                wv_sb[:, dt, :],
                                start=(dt == 0),
                                stop=(dt == NDT - 1),
                            )
                        nc.scalar.copy(
                            v2[:, pt, :, parity, :],
                            vp.rearrange("p (h d) -> p h d", h=H),
                        )
                return v2

            # v projections for the first few pairs can overlap the AllReduce
            v2s = {}
            for pair in range(min(V_PREFETCH, PAIRS)):
                v2s[pair] = emit_v(pair)

            # ---- softmax on the reduced logits (replicated on every core);
            # the 1/sum rescale is folded into the exp tiles themselves ----
            for h in range(H):
                cout = cc_out_a if h < HH else cc_out_b
                dl = dl_pool.tile([P, NJT, N], f32, tag="dl")
                nc.sync.dma_start(dl[:], cout[:, h % HH, :, :])
                for jt in range(NJT):
                    nc.scalar.activation(
                        exp_sb[:, h, jt, :],
                        dl[:, jt, :],
                        AF.Exp,
                        bias=jb_sb[:, jt : jt + 1],
                        scale=1.0,
                    )
                sp = vpsum.tile([1, N], f32, tag="vp")
                for jt in range(NJT):
                    nc.tensor.matmul(
                        sp[:],
                        ones_col[:],
                        exp_sb[:, h, jt, :],
                        start=(jt == 0),
                        stop=(jt == NJT - 1),
                    )
                s_sb = sm_pool.tile([1, N], f32, tag="s_sb")
                nc.any.tensor_copy(out=s_sb[:], in_=sp[:])
                rcp = sm_pool.tile([1, N], f32, tag="rcp")
                nc.vector.reciprocal(rcp[:], s_sb[:])
                rcp_bf = sm_pool.tile([1, N], bf16, tag="rcp_bf")
                nc.any.tensor_copy(out=rcp_bf[:], in_=rcp[:])
                bps = vpsum.tile([P, N], f32, tag="vp")
                nc.tensor.matmul(bps[:], ones_row[:], rcp_bf[:], start=True, stop=True)
                nc.vector.tensor_copy(rs_bcast[:, h, :], bps[:])
                for jt in range(NJT):
                    nc.vector.tensor_tensor(
                        exp_sb[:, h, jt, :],
                        exp_sb[:, h, jt, :],
                        rs_bcast[:, h, :],
                        ALU.mult,
                    )

            # ---- group B: attn @ v, rescale, output projection, store ----
            for pair in range(PAIRS):
                if pair in v2s:
                    v2 = v2s[pair]
                else:
                    v2 = emit_v(pair)
                out2 = [
                    out2_pool.tile([P, NHT, N], bf16, tag="o2a", name=f"o2a_{pair}"),
                    out2_pool.tile([P, NHT, N], bf16, tag="o2b", name=f"o2b_{pair}"),
                ]
                for h in range(H):
                    ap_ps = attpsum.tile([P, N], f32, tag="att")
                    for jt in range(NJT):
                        nc.tensor.matmul(
                            ap_ps[:],
                            v2[:, jt, h, :, :],
                            exp_sb[:, h, jt, :],
                            start=(jt == 0),
                            stop=(jt == NJT - 1),
                        )
                    t, sub = h // 2, h % 2
                    for parity in range(2):
                        o = 64 * parity
                        nc.vector.tensor_copy(
                            out2[parity][sub * 64 : sub * 64 + 64, t, :],
                            ap_ps[o : o + 64, :],
                        )
                for parity in range(2):
                    r = 2 * pair + parity
                    yrow = yrow_pool.tile([P, NPT, D], f32, tag="yrow")
                    for it in range(NPT):
                        yp = attpsum.tile([P, D], f32, tag="att")
                        for t in range(NHT):
                            nc.tensor.matmul(
                                yp[:],
                                out2[parity][:, t, it * P : (it + 1) * P],
                                wo_sb[:, t, :],
                                start=(t == 0),
                                stop=(t == NHT - 1),
                            )
                        if has_bias:
                            nc.vector.tensor_add(
                                out=yrow[:, it, :], in0=yp[:], in1=bo_bcast[:]
                            )
                        else:
                            nc.vector.tensor_copy(yrow[:, it, :], yp[:])
                    nc.sync.dma_start(
                        out_ext[r].rearrange("(po pi) e -> pi po e", pi=P), yrow[:]
                    )

    if do_finalize:
        nc.finalize()
    return nc


def _get_graph(separate_xq: bool, has_bias: bool):
    key = (separate_xq, has_bias)
    if key not in _graph_cache:
        _graph_cache[key] = _build(separate_xq, has_bias)
    return _graph_cache[key]


def _prepare(x, mask, Wq, Wk, Wv, Wo, bo, tie_attn_dim):
    """Host-side prep: mask bookkeeping, weight folding, sharded in_maps."""
    x = np.ascontiguousarray(np.asarray(x, dtype=np.float32))
    mask = np.asarray(mask).astype(bool)
    Wq = np.asarray(Wq, dtype=np.float32)
    Wk = np.ascontiguousarray(np.asarray(Wk, dtype=np.float32))
    Wv = np.ascontiguousarray(np.asarray(Wv, dtype=np.float32))
    Wo = np.ascontiguousarray(np.asarray(Wo, dtype=np.float32))
    bo = np.ascontiguousarray(np.asarray(bo, dtype=np.float32))
    r = int(tie_attn_dim)
    assert x.shape == (B * R, N, D) and r == R, (x.shape, r)

    m = mask.reshape(B, R, N)
    has_rows = m.any(axis=-1)[0]  # [R]
    num_rows = max(int(has_rows.sum()), 1)
    col_valid = m.any(axis=1)[0]  # [N]

    scale = (DH ** -0.5) * (num_rows ** -0.5)
    Wq_eff = np.ascontiguousarray(Wq * np.float32(scale))

    jbias = np.where(col_valid, 0.0, -1e30).astype(np.float32)
    jbias = np.ascontiguousarray(jbias.reshape(NJT, P))

    has_bias = bool(np.any(bo != 0.0))
    separate_xq = not bool(has_rows.all())
    if separate_xq:
        xq = np.ascontiguousarray(x * has_rows[:, None, None].astype(np.float32))
    else:
        xq = None

    in_maps = []
    for c in range(NCORES):
        im = {
            "x": np.ascontiguousarray(x[c * R_LOC : (c + 1) * R_LOC]),
            "Wq": Wq_eff,
            "Wk": Wk,
            "Wv": Wv,
            "Wo": Wo,
            "bo": bo,
            "jbias": jbias,
        }
        if separate_xq:
            im["xq"] = np.ascontiguousarray(xq[c * R_LOC : (c + 1) * R_LOC])
        in_maps.append(im)
    return separate_xq, has_bias, in_maps


def kernel(x, mask, Wq, Wk, Wv, Wo, bo, tie_attn_dim):
    from concourse.bass_utils import run_bass_kernel_spmd

    separate_xq, has_bias, in_maps = _prepare(
        x, mask, Wq, Wk, Wv, Wo, bo, tie_attn_dim
    )
    nc = _get_graph(separate_xq, has_bias)
    res = run_bass_kernel_spmd(nc, in_maps, list(range(NCORES)))
    out = np.concatenate([res.results[c]["out"] for c in range(NCORES)], axis=0)
    return out.astype(np.float32)


def _install_ntff_hook():
    """The agent image's antenv lacks axon_hooks; recreate it so trace=True
    can drive NTFF profiling through libaxon_pjrt.so (see trn_boot.py)."""
    try:
        from antenv import axon_hooks  # noqa: F401

        return
    except ImportError:
        pass
    import types

    import antenv

    mod = types.ModuleType("antenv.axon_hooks")
    holder = {}
    mod.set_axon_ntff_profile_hook = lambda h: holder.__setitem__("h", h)
    mod.get_axon_ntff_profile_hook = lambda: holder.get("h")
    sys.modules["antenv.axon_hooks"] = mod
    antenv.axon_hooks = mod
    if "/root/.axon_site" not in sys.path:
        sys.path.insert(0, "/root/.axon_site")
    from trn_agent_boot.trn_boot import _ntff_profile_via_ctypes

    mod.set_axon_ntff_profile_hook(
        _ntff_profile_via_ctypes("/opt/axon/libaxon_pjrt.so")
    )


def bench(inputs):
    """Run with neuron-profile tracing; returns (exec_time_ns, output)."""
    from concourse.bass_utils import run_bass_kernel_spmd

    _install_ntff_hook()

    separate_xq, has_bias, in_maps = _prepare(**inputs)
    nc = _get_graph(separate_xq, has_bias)
    res = run_bass_kernel_spmd(nc, in_maps, list(range(NCORES)), trace=True)
    out = np.concatenate([res.results[c]["out"] for c in range(NCORES)], axis=0)
    return res, out.astype(np.float32)
